# revision 1
# baseline (speedup 1.0000x reference)
"""Trainium2 Bass kernel for CascadedAttention (Bahdanau attention + GRU recurrence).

Data-parallel over batch across 8 NeuronCores. Per core (B_c=32, T=150, F=1024, U=28):
  Phase 1: UaH^T = (x @ Ua + Ba2)^T  stored bf16 in SBUF as [f_p, fc, b, t];
           XW    = x @ gru_kernel + gru_bias[0], transposed to tau-major for PE.
  Recurrence (150 steps):
           WaS = [h;1] @ [Wa;Ba1]  (PE) -> bf16
           arg = UaH + WaS_bcast   (DVE, bf16)
           th  = tanh(arg)         (ACT)
           scores = Va . th        (PE, col-strip packed PSUM) -> DMA scatter [32,150]
           softmax via max/exp(accum)/recip/tensor_scalar (b on partitions)
           a^T via PE transpose; xz^T[u,b] = sum_tau XW^T[tau,b,u] a[b,tau]  (PE, per-b)
           GRU gates with sigmoid(x) = (1+tanh(x/2))/2 (avoids ACT table switches)
  Output ys^T [U, T, B_c] -> host transpose.

Note: Ba3 is dropped (softmax is shift-invariant); gru_bias[0] is folded into XW
(valid because sum(a)=1); gru_bias[1] is folded into the hz matmul via [h;1].
"""

import os

import numpy as np
import ml_dtypes

import concourse.bass as bass
import concourse.bacc as bacc
import concourse.mybir as mybir
import concourse.tile as tile
from concourse.bass_utils import run_bass_kernel_spmd

BF16 = mybir.dt.bfloat16
F32 = mybir.dt.float32
bf16 = ml_dtypes.bfloat16
AF = mybir.ActivationFunctionType
OP = mybir.AluOpType

B, T, F, U = 256, 150, 1024, 28
NCORES = 8
BC = B // NCORES          # 32 batches per core
N = BC * T                # 4800
KF = F // 128             # 8 f-chunks
U3 = 3 * U                # 84
UP = 96                   # padded gates: z in 0:28, r in 32:60, h in 64:92
KA = 64                   # padded [h;1]: h in rows 0:28, ones in row 32

# score chunks along (b, tau): 16 chunks of 2 batches (300 cols each)
CHUNKS = [(2 * c, 2) for c in range(16)]

_CACHE = {}


def build_nc():
    nc = bacc.Bacc("TRN2", target_bir_lowering=False, debug=False)
    x_t = nc.dram_tensor("x_t", [F, N], BF16, kind="ExternalInput")
    ua = nc.dram_tensor("ua", [F, F], BF16, kind="ExternalInput")
    gk = nc.dram_tensor("gk", [F, UP], BF16, kind="ExternalInput")
    wa = nc.dram_tensor("wa_aug", [KA, F], F32, kind="ExternalInput")
    grk = nc.dram_tensor("grk_aug", [KA, UP], F32, kind="ExternalInput")
    va = nc.dram_tensor("va32", [128, KF, 32], BF16, kind="ExternalInput")
    ba2 = nc.dram_tensor("ba2_cols", [128, KF], F32, kind="ExternalInput")
    gb0 = nc.dram_tensor("gb0", [UP, 1], F32, kind="ExternalInput")
    idf = nc.dram_tensor("id_f32", [128, 128], F32, kind="ExternalInput")
    ys = nc.dram_tensor("ys", [U, T * BC], F32, kind="ExternalOutput")

    with tile.TileContext(nc) as tc:
        with tc.tile_pool(name="persist", bufs=1) as persist:
            uah = persist.tile([128, KF, BC, T], BF16)
            xwt0 = persist.tile([128, BC, UP], F32)   # tau 0:128
            xwt1 = persist.tile([32, BC, UP], F32)    # tau 128:150 in rows 0:22
            ys_sb = persist.tile([U, T, BC], F32)
            wa_sb = persist.tile([KA, F], F32)
            grk_sb = persist.tile([KA, UP], F32)
            va_sb = persist.tile([128, KF, 32], BF16)
            ba2_sb = persist.tile([128, KF], F32)
            gb0_sb = persist.tile([UP, 1], F32)
            idf_sb = persist.tile([128, 128], F32)
            h_aug = persist.tile([KA, BC], F32)

            nc.sync.dma_start(out=wa_sb, in_=wa[:, :])
            nc.sync.dma_start(out=grk_sb, in_=grk[:, :])
            nc.sync.dma_start(out=va_sb, in_=va[:, :])
            nc.sync.dma_start(out=ba2_sb, in_=ba2[:, :])
            nc.sync.dma_start(out=gb0_sb, in_=gb0[:, :])
            nc.sync.dma_start(out=idf_sb, in_=idf[:, :])
            nc.vector.memset(h_aug, 0.0)
            nc.vector.memset(h_aug[32:33, :], 1.0)

            # ---------------- phase 1 ----------------
            with tc.tile_pool(name="ph1w", bufs=1) as ph1w:
                ua_sb = ph1w.tile([128, KF, KF, 128], BF16)  # [k_in_p, kc, fo, m]
                gk_sb = ph1w.tile([128, KF, UP], BF16)
                xw_sb = ph1w.tile([UP, BC, T], F32)
                nc.sync.dma_start(
                    out=ua_sb,
                    in_=ua.rearrange("(kc p) (fo m) -> p kc fo m", p=128, m=128),
                )
                nc.sync.dma_start(
                    out=gk_sb, in_=gk.rearrange("(kc p) u -> p kc u", p=128)
                )
                x3 = x_t.rearrange("f (b t) -> f b t", b=BC)
                with tc.tile_pool(name="ph1x", bufs=16) as ph1x, \
                     tc.tile_pool(name="ph1ps", bufs=4, space="PSUM") as ph1ps, \
                     tc.tile_pool(name="ph1ps2", bufs=2, space="PSUM") as ph1ps2:
                    for b0, nb in CHUNKS:
                        xts = []
                        for kc in range(KF):
                            xt = ph1x.tile([128, 2, T], BF16, tag="xt")
                            nc.sync.dma_start(
                                out=xt[:, 0:nb, :],
                                in_=x3[kc * 128 : (kc + 1) * 128, b0 : b0 + nb, :],
                            )
                            xts.append(xt)
                        for fo in range(KF):
                            ps = ph1ps.tile([128, 2, T], F32, tag="ps")
                            for kc in range(KF):
                                nc.tensor.matmul(
                                    ps[:, 0:nb, :],
                                    ua_sb[:, kc, fo, :],
                                    xts[kc][:, 0:nb, :],
                                    start=(kc == 0),
                                    stop=(kc == KF - 1),
                                )
                            nc.scalar.activation(
                                uah[:, fo, b0 : b0 + nb, :],
                                ps[:, 0:nb, :],
                                AF.Identity,
                                bias=ba2_sb[:, fo : fo + 1],
                            )
                        ps2 = ph1ps2.tile([UP, 2, T], F32, tag="ps2")
                        for kc in range(KF):
                            nc.tensor.matmul(
                                ps2[:, 0:nb, :],
                                gk_sb[:, kc, :],
                                xts[kc][:, 0:nb, :],
                                start=(kc == 0),
                                stop=(kc == KF - 1),
                            )
                        nc.scalar.activation(
                            xw_sb[:, b0 : b0 + nb, :],
                            ps2[:, 0:nb, :],
                            AF.Identity,
                            bias=gb0_sb[:, 0:1],
                        )
                # transpose XW -> tau-major
                with tc.tile_pool(name="trps", bufs=2, space="PSUM") as trps:
                    for b in range(BC):
                        p0 = trps.tile([128, UP], F32, tag="tr0")
                        nc.tensor.transpose(
                            p0, xw_sb[:, b, 0:128], idf_sb[0:UP, 0:UP]
                        )
                        nc.vector.tensor_copy(xwt0[:, b, :], p0)
                        p1 = trps.tile([32, UP], F32, tag="tr1")
                        nc.tensor.transpose(
                            p1[0:22, :], xw_sb[:, b, 128:T], idf_sb[0:UP, 0:UP]
                        )
                        nc.vector.tensor_copy(xwt1[0:22, b, :], p1[0:22, :])

            # ---------------- recurrence ----------------
            with tc.tile_pool(name="rec2", bufs=3) as rec2, \
                 tc.tile_pool(name="recs", bufs=2) as recs, \
                 tc.tile_pool(name="ps_sc", bufs=4, space="PSUM") as ps_sc, \
                 tc.tile_pool(name="ps_was", bufs=1, space="PSUM") as ps_was, \
                 tc.tile_pool(name="ps_xz", bufs=1, space="PSUM") as ps_xz, \
                 tc.tile_pool(name="ps_hz", bufs=1, space="PSUM") as ps_hz, \
                 tc.tile_pool(name="ps_tr", bufs=1, space="PSUM") as ps_tr:
                for t in range(int(os.environ.get("KSTEPS", T))):
                    # WaS^T[f, b] = [Wa;Ba1]^T [h;1]
                    wps = ps_was.tile([128, KF, BC], F32, tag="wps")
                    for fc in range(KF):
                        nc.tensor.matmul(
                            wps[:, fc, :],
                            wa_sb[:, fc * 128 : (fc + 1) * 128],
                            h_aug,
                            start=True,
                            stop=True,
                        )
                    # hz^T = [grk; gb1]^T [h;1] -- issued early so its waits
                    # coalesce with the WaS matmuls (same h_aug dependency)
                    hzp = ps_hz.tile([UP, BC], F32, tag="hzp")
                    nc.tensor.matmul(hzp, grk_sb, h_aug, start=True, stop=True)
                    was_sb = recs.tile([128, KF, BC], BF16, tag="was")
                    nc.scalar.activation(was_sb, wps, AF.Copy)

                    sc_tiles = [
                        ps_sc.tile([128, 300], F32, tag="sc", name=f"sc{g}")
                        for g in range(4)
                    ]
                    for fc in range(KF):
                        add_t = rec2.tile([128, BC, T], BF16, tag="add")
                        nc.vector.tensor_tensor(
                            add_t,
                            uah[:, fc],
                            was_sb[:, fc, :].broadcast_to([128, BC, T]),
                            op=OP.add,
                        )
                        th_t = rec2.tile([128, BC, T], BF16, tag="th")
                        nc.scalar.activation(th_t, add_t, AF.Tanh)
                        for ci, (b0, nb) in enumerate(CHUNKS):
                            g, j = divmod(ci, 4)
                            nc.tensor.matmul(
                                sc_tiles[g][32 * j : 32 * j + 32, 0 : nb * T],
                                va_sb[:, fc, :],
                                th_t[:, b0 : b0 + nb, :],
                                start=(fc == 0),
                                stop=(fc == KF - 1),
                                tile_position=(0, 32 * j),
                                skip_group_check=True,
                            )
                    # each chunk's scores are replicated across its 32-row
                    # strip; full-width copies PSUM->SBUF, then DMA-scatter
                    sc_sb = recs.tile([128, 4, 300], F32, tag="sc_sb")
                    for g in range(4):
                        nc.vector.tensor_copy(sc_sb[:, g, :], sc_tiles[g][:, :])
                    scores32 = recs.tile([BC, T], F32, tag="sc32")
                    for ci in range(16):
                        g, j = divmod(ci, 4)
                        nc.sync.dma_start(
                            out=scores32[2 * ci : 2 * ci + 2, :],
                            in_=sc_sb[32 * j : 32 * j + 1, g, :],
                        )
                    # softmax (b on partitions)
                    m32 = recs.tile([BC, 1], F32, tag="m32")
                    nc.vector.reduce_max(m32, scores32, axis=mybir.AxisListType.X)
                    nm32 = recs.tile([BC, 1], F32, tag="nm32")
                    nc.vector.tensor_scalar_mul(nm32, m32, -1.0)
                    e32 = recs.tile([BC, T], F32, tag="e32")
                    s32 = recs.tile([BC, 1], F32, tag="s32")
                    nc.scalar.activation(
                        e32, scores32, AF.Exp, bias=nm32[:, 0:1], accum_out=s32
                    )
                    r32 = recs.tile([BC, 1], F32, tag="r32")
                    nc.vector.reciprocal(r32, s32)
                    a32 = recs.tile([BC, T], F32, tag="a32")
                    nc.vector.tensor_scalar_mul(a32, e32, r32[:, 0:1])
                    # a^T via PE transpose (both tau-chunks share one PSUM bank)
                    trc = ps_tr.tile([128, 2, BC], F32, tag="trc")
                    nc.tensor.transpose(
                        trc[:, 0, :], a32[:, 0:128], idf_sb[0:BC, 0:BC]
                    )
                    nc.tensor.transpose(
                        trc[0:22, 1, :], a32[:, 128:T], idf_sb[0:BC, 0:BC]
                    )
                    at0 = recs.tile([128, BC], F32, tag="at0")
                    nc.vector.tensor_copy(at0, trc[:, 0, :])
                    at1 = recs.tile([32, BC], F32, tag="at1")
                    nc.vector.tensor_copy(at1[0:22, :], trc[0:22, 1, :])
                    # xz^T[u, b] (PE, per-b matvec over tau)
                    xzp = ps_xz.tile([UP, BC], F32, tag="xzp")
                    for b in range(BC):
                        nc.tensor.matmul(
                            xzp[:, b : b + 1],
                            xwt0[:, b, :],
                            at0[:, b : b + 1],
                            start=True,
                            stop=False,
                        )
                        nc.tensor.matmul(
                            xzp[:, b : b + 1],
                            xwt1[0:22, b, :],
                            at1[0:22, b : b + 1],
                            start=False,
                            stop=True,
                        )
                    bh = recs.tile([32, BC], F32, tag="bh")
                    nc.vector.tensor_copy(bh, hzp[64:96, :])
                    bzr = recs.tile([64, BC], F32, tag="bzr")
                    nc.vector.tensor_copy(bzr, hzp[0:64, :])
                    # z,r = sigmoid(xz+hz) = 0.5*(1+tanh(0.5*(xz+hz)))
                    g_sb = recs.tile([64, BC], F32, tag="gsb")
                    nc.vector.tensor_add(g_sb, xzp[0:64, :], bzr)
                    tzr = recs.tile([64, BC], F32, tag="tzr")
                    nc.scalar.activation(tzr, g_sb, AF.Tanh, scale=0.5)
                    trr = recs.tile([32, BC], F32, tag="trr")
                    nc.vector.tensor_copy(trr, tzr[32:64, :])
                    # hh = tanh(x_h + r*hz_h);  r*hz_h = 0.5*(hz_h + tz_r*hz_h)
                    v_sb = recs.tile([32, BC], F32, tag="vsb")
                    nc.vector.tensor_mul(v_sb, trr, bh)
                    w_sb = recs.tile([32, BC], F32, tag="wsb")
                    nc.vector.tensor_add(w_sb, bh, v_sb)
                    ti_sb = recs.tile([32, BC], F32, tag="tisb")
                    nc.vector.scalar_tensor_tensor(
                        ti_sb, w_sb, 0.5, xzp[64:96, :], OP.mult, OP.add
                    )
                    hh = recs.tile([32, BC], F32, tag="hh")
                    nc.scalar.activation(hh, ti_sb, AF.Tanh)
                    # h_new = hh + z*(h-hh) = hh + 0.5*(1+tz_z)*(h-hh)
                    t1 = recs.tile([32, BC], F32, tag="t1")
                    nc.vector.tensor_sub(t1, h_aug[0:32, :], hh)
                    p_sb = recs.tile([32, BC], F32, tag="psb")
                    nc.vector.tensor_mul(p_sb, tzr[0:32, :], t1)
                    q_sb = recs.tile([32, BC], F32, tag="qsb")
                    nc.vector.tensor_add(q_sb, t1, p_sb)
                    nc.vector.scalar_tensor_tensor(
                        ys_sb[:, t, :], q_sb[0:U, :], 0.5, hh[0:U, :], OP.mult, OP.add
                    )
                    nc.vector.tensor_copy(h_aug[0:U, :], ys_sb[:, t, :])

            nc.sync.dma_start(
                out=ys[:, :], in_=ys_sb.rearrange("u t b -> u (t b)")
            )
    nc.compile()
    return nc


def _pad_gates(w):
    """(K, 84) -> (K, 96): z cols at 0:28, r at 32:60, h at 64:92, zeros else."""
    w = np.asarray(w)
    out = np.zeros(w.shape[:-1] + (UP,), w.dtype)
    for i in range(3):
        out[..., 32 * i : 32 * i + U] = w[..., U * i : U * (i + 1)]
    return out


def _pad_h(w, last_row):
    """(28, N) + (N,) bias -> (64, N): w in rows 0:28, bias in row 32."""
    out = np.zeros((KA,) + w.shape[1:], np.float32)
    out[0:U] = w
    out[32] = last_row
    return out


def _prep_inputs(x, Wa, Ua, Va, Ba1, Ba2, Ba3, gru_kernel, gru_rkernel, gru_bias):
    shared = {
        "ua": np.ascontiguousarray(Ua.astype(bf16)),
        "gk": np.ascontiguousarray(_pad_gates(gru_kernel).astype(bf16)),
        "wa_aug": np.ascontiguousarray(_pad_h(Wa, Ba1[0]).astype(np.float32)),
        "grk_aug": np.ascontiguousarray(
            _pad_h(_pad_gates(gru_rkernel), _pad_gates(gru_bias[1:2])[0]).astype(
                np.float32
            )
        ),
        "va32": np.ascontiguousarray(
            np.repeat(Va[:, 0].reshape(KF, 128).T[:, :, None], 32, axis=2).astype(
                bf16
            )
        ),
        "ba2_cols": np.ascontiguousarray(
            Ba2[0].reshape(KF, 128).T.astype(np.float32)
        ),
        "gb0": np.ascontiguousarray(
            _pad_gates(gru_bias[0:1])[0].reshape(UP, 1).astype(np.float32)
        ),
        "id_f32": np.eye(128, dtype=np.float32),
    }
    x_bf = x.astype(bf16)  # single pass over the fp32 data
    in_maps = []
    for c in range(NCORES):
        xc = x_bf[c * BC : (c + 1) * BC]  # (BC, T, F) bf16
        x_t = np.ascontiguousarray(xc.transpose(2, 0, 1).reshape(F, N))
        in_maps.append({"x_t": x_t, **shared})
    return in_maps


def _run(inputs, trace=False, **kw):
    if "nc" not in _CACHE:
        _CACHE["nc"] = build_nc()
    nc = _CACHE["nc"]
    in_maps = _prep_inputs(**inputs)
    res = run_bass_kernel_spmd(nc, in_maps, list(range(NCORES)), trace=trace, **kw)
    outs = []
    for c in range(NCORES):
        y = res.results[c]["ys"].reshape(U, T, BC).transpose(2, 1, 0)
        outs.append(y)
    return np.ascontiguousarray(np.concatenate(outs, axis=0).astype(np.float32)), res


def kernel(**inputs):
    out, _ = _run(inputs, trace=False)
    return out

